# revision 51
# baseline (speedup 1.0000x reference)
"""Trainium2 Bass kernel for nn_Encoder (dense MLP with stochastic ternarization).

y = tanh(x @ (s1*T(w1,n1)) + b1) @ (s2*T(w2,n2)) + b2,  T(w,n) = (w-n>1) - (w-n<-1)

Sharding: tensor-parallel over the 16384 hidden dim across 8 cores. Each core
gets a 2048-wide hidden shard of w1/noise1/s1/b1 (column-sharded) and the
matching 2048-row shard of w2/noise2; x is replicated (host pre-transposed to
bf16). Each core computes partial yT = (h_shard @ w2_shard).T in bf16 per
512-batch block; per-block, per-256-row quarter ReduceScatters(add) hand core c
rows {256q+32c .. +32} where s2/b2 are applied. The host reassembles.

Final structure (~630us HW; baseline was ~650us):
- Supply is column-major at 256KB granularity (piece = 4 k-tiles x 128 cols);
  8-deep stage buffering keeps the DMA->DVE-sub->2xACT-tanh->DVE-add ternarize
  pipeline throughput-bound (~1.8us/piece) instead of latency-bound, and the
  t2g add is lagged one piece in DVE issue order so the DVE queue never
  serializes the pipeline.
- Phase 1 runs blocks 0/1 with block 1 staggered 4 columns behind block 0, so
  x1's DMA is off the critical prefix; first matmul issues ~20us in.
- w2 ternarize is spread over the last 12 phase-1 slots (after all w1 terns
  are issued, to avoid a cross-queue WAR cycle), so t22 is ready when L2(b0)
  starts right after phase 1.
- Layer-2 partials get s2/b2 folded into a bf16 cast and are stored
  row-permuted so ONE ReduceScatter per block yields each core its final
  rows; the post-collective path is a single upcast (sync load + DVE + store).
- PE runs at the machine's power-capped rate (~260-275ns per 512-wide bf16
  matmul; GPIO 13/16 throttle or P0 2.0GHz downclock) with <40us of gaps.
"""

import sys

for _p in ("/opt/trn_rl_repo",):
    if _p not in sys.path:
        sys.path.insert(0, _p)

import numpy as np
import ml_dtypes

import concourse.bass as bass
import concourse.bacc as bacc
import concourse.mybir as mybir
import concourse.tile as tile
from concourse.bass_utils import run_bass_kernel_spmd

BF16 = mybir.dt.bfloat16
F32 = mybir.dt.float32
FP8 = mybir.dt.float8e4
NPBF16 = ml_dtypes.bfloat16

N_CORES = 8
B = 2048
DIN = 3072
DHID = 16384
DOUT = 1024
HSH = DHID // N_CORES    # 2048
K1 = DIN // 128          # 24 contraction tiles, layer 1
NP1 = 3                  # x pieces per block (8 k-tiles each)
KP = K1 // NP1           # 8 k-tiles per x piece
NPW = 6                  # w1/n1 supply pieces per column (4 k-tiles each)
KPW = K1 // NPW          # 4 k-tiles per weight piece
K2 = HSH // 128          # 16 contraction tiles, layer 2
NB = B // 512            # 4 batch blocks
MT = HSH // 128          # 16 hidden columns (128 wide)
ND = DOUT // 128         # 8 dout tiles
HCHUNK = 64              # rows per core per half, baseline row mapping
HROWS = DOUT // 2        # 512

BIGK = float(2 ** 30)

TANH = mybir.ActivationFunctionType.Tanh
MULT = mybir.AluOpType.mult
ADD = mybir.AluOpType.add


def build_bass():
    nc = bacc.Bacc("TRN2", target_bir_lowering=False, debug=False, num_devices=N_CORES)

    # x: [block, piece, 128 part(k-in), 8 k, 512 batch] bf16
    xtb = nc.dram_tensor("xtb", [NB, NP1, 128, KP, 512], BF16, kind="ExternalInput")
    # w1/n1: [col, piece, 128 part(k-in), 4 k, 128 m] f32
    w1p = nc.dram_tensor("w1p", [MT, NPW, 128, KPW, 128], F32, kind="ExternalInput")
    n1p = nc.dram_tensor("n1p", [MT, NPW, 128, KPW, 128], F32, kind="ExternalInput")
    s1h = nc.dram_tensor("s1h", [128, MT], F32, kind="ExternalInput")
    b1m = nc.dram_tensor("b1m", [128, MT], F32, kind="ExternalInput")
    # w2/n2: [k2, half, 128 part(hid-in), 4 dtile, 128 dout] f32
    w2p = nc.dram_tensor("w2p", [K2, 2, 128, KPW, 128], F32, kind="ExternalInput")
    n2p = nc.dram_tensor("n2p", [K2, 2, 128, KPW, 128], F32, kind="ExternalInput")
    # s2 (x0.5) per dout tile; b2/8 per dout tile (applied pre-reduce)
    s2f = nc.dram_tensor("s2f", [128, ND], F32, kind="ExternalInput")
    b2f = nc.dram_tensor("b2f", [128, ND], F32, kind="ExternalInput")

    # partition p of yTc = global dout row 256*(p//32) + 32*core + (p%32)
    yTc = nc.dram_tensor("yTc", [128, B], F32, kind="ExternalOutput")

    with tile.TileContext(nc) as tc:
        with (
            tc.tile_pool(name="const", bufs=1) as cpool,
            tc.tile_pool(name="dram", bufs=1, space="DRAM") as dpool,
            tc.tile_pool(name="t2w1", bufs=MT) as t2pool,
            tc.tile_pool(name="t2w2", bufs=1) as t22pool,
            tc.tile_pool(name="stage", bufs=2) as spool,
            tc.tile_pool(name="xtn", bufs=7) as xpool,
            tc.tile_pool(name="hblk", bufs=40) as hpool,
            tc.tile_pool(name="yblk", bufs=4) as ypool,
            tc.tile_pool(name="fin", bufs=2) as fpool,
            tc.tile_pool(name="ps1", bufs=6, space="PSUM") as pspool,
            tc.tile_pool(name="ps2", bufs=2, space="PSUM") as ps2pool,
        ):
            kneg = cpool.tile([128, 1], F32, tag="kneg")
            nc.vector.memset(kneg[:], -BIGK)
            kpos = cpool.tile([128, 1], F32, tag="kpos")
            nc.vector.memset(kpos[:], BIGK)

            # per-block partial / scattered buffers (bf16). The partials are
            # stored row-PERMUTED so a single ReduceScatter hands core c rows
            # {64c..64c+64} and {512+64c..+64} (the baseline mapping).
            yT_b = [dpool.tile([DOUT, 512], BF16, tag=f"yTp{b}",
                               name=f"yT_b{b}") for b in range(NB)]
            rs_b = [dpool.tile([2 * HCHUNK, 512], BF16, tag=f"rs{b}",
                               name=f"rs_b{b}") for b in range(NB)]

            # x sub-tiles: [128, 8, 512] bf16; allocate all 12 upfront so the
            # 8-slot cycling is fixed (x2p0/x2p1 land in fresh slots and can
            # prefetch; the rest recycle x0/x1 slots as phase 1 retires)
            xs = {(b, p): xpool.tile([128, KP, 512], BF16, tag="xs",
                                     name=f"xs{b}_{p}")
                  for b in range(NB) for p in range(NP1)}

            def x_load(b, p, engine):
                engine.dma_start(xs[(b, p)][:], xtb[b, p])

            t2g = [t2pool.tile([128, K1, 128], FP8, tag="t2", name=f"t2c{c}")
                   for c in range(MT)]
            t22 = t22pool.tile([128, K2, KP, 128], FP8, tag="t22")

            # ---- supply DMAs ----
            # sync ring: x0 pieces just-in-time interleaved with column 0,
            # then columns in consumption order; x1 after column 5; w2 last.
            # scalar ring: x2/x3 prefetch (WAR-gated on slot recycling).
            x_load(0, 0, nc.sync)

            w_stage = []

            def w1_load(c):
                for p in range(NPW):
                    w_t = spool.tile([128, KPW, 128], F32, tag="w", bufs=8,
                                     name=f"w1s_{c}_{p}")
                    nc.sync.dma_start(w_t[:], w1p[c, p])
                    n_t = spool.tile([128, KPW, 128], F32, tag="n", bufs=8,
                                     name=f"n1s_{c}_{p}")
                    nc.sync.dma_start(n_t[:], n1p[c, p])
                    w_stage.append((w_t, n_t))
                    if c == 0 and p in (1, 3):
                        x_load(0, p // 2 + 1, nc.sync)

            s1_sb = cpool.tile([128, MT], F32, tag="s1")
            b1_sb = cpool.tile([128, MT], F32, tag="b1")
            s2_sb = cpool.tile([128, ND], F32, tag="s2")
            b2_sb = cpool.tile([128, ND], F32, tag="b2")

            for c in range(MT):
                w1_load(c)
                if c == 1:
                    nc.sync.dma_start(s1_sb[:], s1h[:, :])
                    nc.sync.dma_start(b1_sb[:], b1m[:, :])
                    nc.sync.dma_start(s2_sb[:], s2f[:, :])
                    nc.sync.dma_start(b2_sb[:], b2f[:, :])
                if c == 1:
                    x_load(1, 0, nc.sync)
                if c == 2:
                    x_load(1, 1, nc.sync)
                    x_load(1, 2, nc.sync)
            # x2p0 lands right at the phase-2 boundary via the sync ring; the
            # rest of x2/x3 WAR-gate on phase-1 slot recycling (scalar ring)
            x_load(2, 0, nc.sync)
            x_load(2, 1, nc.scalar)
            x_load(2, 2, nc.scalar)
            for p in range(NP1):
                x_load(3, p, nc.scalar)
            w2_stage = []
            for k2 in range(K2):
                for hh in range(2):
                    w_t = spool.tile([128, KPW, 128], F32, tag="w", bufs=8,
                                     name=f"w2s_{k2}_{hh}")
                    nc.sync.dma_start(w_t[:], w2p[k2, hh])
                    n_t = spool.tile([128, KPW, 128], F32, tag="n", bufs=8,
                                     name=f"n2s_{k2}_{hh}")
                    nc.sync.dma_start(n_t[:], n2p[k2, hh])
                    w2_stage.append((w_t, n_t))

            # ---- ternarize steps (DVE + ACT); ACT program order interleaves
            # the h activations a few columns behind the tern supply.
            # The t2g-add is LAGGED one piece in DVE issue order: add(i)
            # waits on ACT(i), so putting it after sub(i+1) keeps the DVE
            # queue from serializing the whole piece pipeline. ----
            _tern_pending = []

            def _tern_flush():
                while _tern_pending:
                    dst_ap, a1, a2 = _tern_pending.pop(0)
                    nc.vector.tensor_add(dst_ap, a1[:], a2[:])

            def tern_piece(dst_ap, w_t, n_t, name):
                nc.vector.tensor_sub(w_t[:], w_t[:], n_t[:])
                a1 = spool.tile([128, KPW, 128], FP8, tag="a1", bufs=4,
                                name=f"a1_{name}")
                nc.scalar.activation(a1[:], w_t[:], TANH, bias=kneg[:, 0:1],
                                     scale=BIGK)
                a2 = spool.tile([128, KPW, 128], FP8, tag="a2", bufs=4,
                                name=f"a2_{name}")
                nc.scalar.activation(a2[:], w_t[:], TANH, bias=kpos[:, 0:1],
                                     scale=BIGK)
                _tern_pending.append((dst_ap, a1, a2))
                while len(_tern_pending) > 2:
                    d, x, y = _tern_pending.pop(0)
                    nc.vector.tensor_add(d, x[:], y[:])

            h_tiles = {}

            def h_act(b, c, ps):
                h_t = hpool.tile([128, 512], BF16, tag="h", name=f"h{b}_{c}")
                nc.scalar.activation(h_t[:], ps[:], TANH,
                                     bias=b1_sb[:, c:c + 1],
                                     scale=s1_sb[:, c:c + 1])
                h_tiles[(b, c)] = h_t

            # layer-1 chain for one (block, column) -> PSUM (not yet activated)
            def l1_chain(b, c):
                ps = pspool.tile([128, 512], F32, tag="ps", name=f"ps{b}_{c}")
                for k in range(K1):
                    nc.tensor.matmul(
                        ps[:],
                        t2g[c][:, k, :],
                        xs[(b, k // KP)][:, k % KP, :],
                        start=(k == 0), stop=(k == K1 - 1))
                return ps

            # tern for column c (6 pieces)
            def tern_col(c):
                for p in range(NPW):
                    w_t, n_t = w_stage[c * NPW + p]
                    tern_piece(t2g[c][:, p * KPW:(p + 1) * KPW, :], w_t, n_t,
                               f"c{c}p{p}")

            TERN_LEAD = 4
            STAG = 4
            # phase 1: block 1 staggered STAG columns behind block 0 (so x1
            # is off the critical DMA prefix); tern issued TERN_LEAD columns
            # ahead of block 0 on the ACT queue
            for c in range(TERN_LEAD):
                tern_col(c)
            seq = []
            for c in range(MT + STAG):
                if c < MT:
                    seq.append((0, c))
                if c >= STAG:
                    seq.append((1, c - STAG))
            w2_pieces = [(k2, hh) for k2 in range(K2) for hh in range(2)]

            def tern_w2(n):
                while n and w2_pieces:
                    k2, hh = w2_pieces.pop(0)
                    w_t, n_t = w2_stage[k2 * 2 + hh]
                    tern_piece(t22[:, k2, hh * KPW:(hh + 1) * KPW, :],
                               w_t, n_t, f"w2k{k2}h{hh}")
                    n -= 1

            for i, (b, c) in enumerate(seq):
                if b == 0 and c + TERN_LEAD < MT:
                    tern_col(c + TERN_LEAD)
                if i >= len(seq) - 12:
                    tern_w2(3)
                ps = l1_chain(b, c)
                if i >= len(seq) - 8:
                    # ACT is busy with w2 terns here: free the PSUM bank via
                    # a DVE copy and run the tanh from the bf16 scratch
                    hraw = ypool.tile([128, 512], BF16, tag="y",
                                      name=f"hr{b}_{c}")
                    nc.vector.tensor_copy(hraw[:], ps[:])
                    h_act(b, c, hraw)
                else:
                    h_act(b, c, ps)
            _tern_flush()

            # layer-2 for one block: 8 dout chains, bf16 partials stored
            # row-permuted, one ReduceScatter per block
            def layer2_block(b):
                for d in range(ND):
                    p2 = ps2pool.tile([128, 512], F32, tag="ps2",
                                      name=f"ps2_{b}_{d}")
                    for k2 in range(K2):
                        nc.tensor.matmul(p2[:], t22[:, k2, d, :],
                                         h_tiles[(b, k2)][:],
                                         start=(k2 == 0), stop=(k2 == K2 - 1))
                    # s2*y + b2/8 folded into the bf16 cast so the RS output
                    # is final (up to the fp32 cast-DMA)
                    y_sb = ypool.tile([128, 512], BF16, tag="y",
                                      name=f"y{b}_{d}")
                    nc.vector.tensor_scalar(
                        y_sb[:], p2[:], s2_sb[:, d:d + 1], b2_sb[:, d:d + 1],
                        MULT, ADD,
                    )
                    # permuted store: RS chunk c then holds dout rows
                    # {64c..64c+64} u {512+64c..+64} (baseline mapping)
                    base = 256 * (d % 4) + 64 * (d // 4)
                    nc.sync.dma_start(yT_b[b][base:base + 64, :],
                                      y_sb[0:64, :])
                    nc.sync.dma_start(yT_b[b][base + 128:base + 192, :],
                                      y_sb[64:128, :])
                nc.gpsimd.collective_compute(
                    "ReduceScatter",
                    mybir.AluOpType.add,
                    replica_groups=[list(range(N_CORES))],
                    ins=[yT_b[b].opt()],
                    outs=[rs_b[b].opt()],
                )

            # post-RS upcast bf16 -> fp32 on sync + DVE (keeps the gpsimd
            # queue free so the next RS triggers immediately)
            def fin_block(b):
                rs_sb = fpool.tile([128, 512], BF16, tag="rsb", name=f"rsb{b}")
                nc.sync.dma_start(rs_sb[:], rs_b[b][:, :])
                out_sb = fpool.tile([128, 512], F32, tag="osb", name=f"osb{b}")
                nc.vector.tensor_copy(out_sb[:], rs_sb[:])
                nc.sync.dma_start(yTc[:, b * 512:(b + 1) * 512], out_sb[:])

            # phase 2: w2/t22 is ready early (supply finishes ~110us), so the
            # layer-2 blocks run as soon as their h completes; RS's spread out
            layer2_block(0)
            layer2_block(1)
            fin_block(0)
            for c in range(MT):
                ps = l1_chain(2, c)
                h_act(2, c, ps)
            layer2_block(2)
            fin_block(1)
            for c in range(MT):
                ps = l1_chain(3, c)
                h_act(3, c, ps)
            layer2_block(3)
            fin_block(2)
            fin_block(3)

    nc.compile()
    return nc


_NC_CACHE = {}


def _get_nc():
    if "nc" not in _NC_CACHE:
        _NC_CACHE["nc"] = build_bass()
    return _NC_CACHE["nc"]


def _make_in_maps(x, w1, s1, b1, w2, s2, b2, noise1, noise2):
    x = np.asarray(x, dtype=np.float32)
    w1 = np.asarray(w1, dtype=np.float32)
    s1 = np.asarray(s1, dtype=np.float32)
    b1 = np.asarray(b1, dtype=np.float32)
    w2 = np.asarray(w2, dtype=np.float32)
    s2 = np.asarray(s2, dtype=np.float32)
    b2 = np.asarray(b2, dtype=np.float32)
    noise1 = np.asarray(noise1, dtype=np.float32)
    noise2 = np.asarray(noise2, dtype=np.float32)

    xT = x.T.astype(NPBF16)  # [3072, 2048]
    # -> [NB, NP1, 128, KP, 512]
    xtb = np.ascontiguousarray(
        xT.reshape(NP1, KP, 128, NB, 512).transpose(3, 0, 2, 1, 4))

    def w1_tile(w):   # [din, HSH] -> [MT, NPW, 128, KPW, 128]
        return np.ascontiguousarray(
            w.reshape(NPW, KPW, 128, MT, 128).transpose(3, 0, 2, 1, 4))

    def w2_tile(w):   # [HSH, DOUT] -> [K2, 2, 128, KPW, 128]
        return np.ascontiguousarray(
            w.reshape(K2, 128, 2, KPW, 128).transpose(0, 2, 1, 3, 4))

    # core c, partition p -> dout row 64c + p (p<64) / 512 + 64c + (p-64)
    rows_per_core = []
    for c in range(N_CORES):
        rows = np.concatenate([
            np.arange(HCHUNK * c, HCHUNK * (c + 1)),
            np.arange(HROWS + HCHUNK * c, HROWS + HCHUNK * (c + 1))])
        rows_per_core.append(rows)

    # s2 (x0.5 for the ternary doubling), b2/8 (summed across the 8 ranks)
    s2f = np.ascontiguousarray((0.5 * s2).reshape(ND, 128).T)
    b2f = np.ascontiguousarray((b2 / N_CORES).reshape(ND, 128).T)

    in_maps = []
    for c in range(N_CORES):
        hs = slice(c * HSH, (c + 1) * HSH)
        in_maps.append({
            "xtb": xtb,
            "w1p": w1_tile(w1[:, hs]),
            "n1p": w1_tile(noise1[:, hs]),
            "s1h": np.ascontiguousarray((0.5 * s1[hs]).reshape(MT, 128).T),
            "b1m": np.ascontiguousarray(b1[hs].reshape(MT, 128).T),
            "w2p": w2_tile(np.ascontiguousarray(w2[hs, :])),
            "n2p": w2_tile(np.ascontiguousarray(noise2[hs, :])),
            "s2f": s2f,
            "b2f": b2f,
        })
    return in_maps, rows_per_core


def kernel(x, w1, s1, b1, w2, s2, b2, noise1, noise2, _bench_out=None):
    """Full-input, full-output entry point. Shards across 8 NeuronCores."""
    nc = _get_nc()
    in_maps, rows_per_core = _make_in_maps(
        x, w1, s1, b1, w2, s2, b2, noise1, noise2)
    res = run_bass_kernel_spmd(nc, in_maps, core_ids=list(range(N_CORES)))
    if _bench_out is not None:
        _bench_out.append(res)
    yT = np.empty((DOUT, B), dtype=np.float32)
    for c in range(N_CORES):
        yT[rows_per_core[c], :] = res.results[c]["yTc"]
    return np.ascontiguousarray(yT.T).astype(np.float32)


if __name__ == "__main__":
    nc = build_bass()
    print("built OK")
